# revision 41
# baseline (speedup 1.0000x reference)
"""Trainium2 Bass kernel for DecoderAttnRNN (LSTM + attention decoder).

Sharding: hybrid over 8 cores = 4 batch-groups x 2 vocab-halves; each core
runs 16 batches x 16000 vocab columns.

Single fused pipeline in t-major layout (flat row = t*16 + b):
  - LSTM recurrence runs in transposed layout (gate features on partitions,
    batch on the free dim).  All four gates go through ONE tanh activation
    per step: sigma(x) = 0.5*(tanh(x/2)+1), with the 0.5's folded into the
    host-side weight prep (carry is stored as 2c, hidden state as 2h).
    x@W_ih.T + bias is precomputed and added into PSUM with an identity
    matmul, so the per-step chain is PE -> Act -> DVE x3 -> Act -> DVE.
  - Attention for each 8-step block runs right after the block's last step.
  - The 512x16000 logits GEMM for block m is interleaved into block m+1's
    LSTM steps (2 x 1000-col groups per step) so the PE array streams
    continuously instead of idling during the recurrence.
  - Logits are written to DRAM as bf16 (halves output traffic); b_lin is
    added on the host during unsharding.
"""

import numpy as np
import ml_dtypes

import concourse.bass as bass
import concourse.mybir as mybir
import concourse.tile as tile
from concourse import bacc
from concourse.bass_utils import run_bass_kernel_spmd
from concourse.masks import make_identity

B, T, S, E, H, V = 64, 72, 72, 128, 256, 32000
NCORES = 8
NBG = 4                   # batch groups
NVH = 2                   # vocab halves
BL = B // NBG             # 16 batches per core
BT = BL * T               # 1152 (t-major: flat index = t*BL + b)
VL = V // NVH             # 16000 vocab cols per core
G4H = 4 * H               # 1024
NCH = G4H // 128          # 8 gate chunks of 128
TB = 8                    # timesteps per block
NBLK = T // TB            # 9 blocks == 9 GEMM m-tiles
NGRP = 16                 # GEMM column groups per m-tile (1000 cols each)
GC = VL // NGRP           # 1000
NC = GC // 2              # 500 cols per PSUM-bank chunk

f32 = mybir.dt.float32
bf16 = mybir.dt.bfloat16
i32 = mybir.dt.int32

_CACHE = {}


def _build():
    nc = bacc.Bacc(None, target_bir_lowering=False)

    xT_d = nc.declare_dram_parameter("xT", [E, BT], bf16, isOutput=False)
    enc_d = nc.declare_dram_parameter("enc", [S, BL, H], bf16, isOutput=False)
    encT_d = nc.declare_dram_parameter("encT", [2, 128, BL, S], bf16, isOutput=False)
    h0T_d = nc.declare_dram_parameter("h0T", [128, 2, BL], f32, isOutput=False)
    c0T_d = nc.declare_dram_parameter("c0T", [128, 2, BL], f32, isOutput=False)
    lens_d = nc.declare_dram_parameter("lens", [BL], i32, isOutput=False)
    biasT_d = nc.declare_dram_parameter("biasT", [128, NCH], f32, isOutput=False)
    wihT_d = nc.declare_dram_parameter("wihT", [E, G4H], bf16, isOutput=False)
    whhT_d = nc.declare_dram_parameter("whhT", [2, 128, G4H], bf16, isOutput=False)
    wlinT_d = nc.declare_dram_parameter("wlinT", [4, 128, VL], bf16, isOutput=False)
    out_d = nc.declare_dram_parameter("logits", [BT, VL], bf16, isOutput=True)

    with tile.TileContext(nc) as tc:
        with (
            tc.tile_pool(name="persist", bufs=1) as pp,
            tc.tile_pool(name="work", bufs=2) as wp,
            tc.tile_pool(name="evict", bufs=4) as ep,
            tc.tile_pool(name="psg", bufs=1, space="PSUM") as psg,
            tc.tile_pool(name="psa", bufs=1, space="PSUM") as psa,
            tc.tile_pool(name="psgm", bufs=3, space="PSUM") as psgm,
        ):
            # ---- small persistent inputs, all on the sync queue BEFORE the
            # W_lin stream so they are not starved behind 16 MB of weights ----
            x_allT = pp.tile([128, BT], bf16)         # embeddings^T, (t,b) cols
            nc.sync.dma_start(out=x_allT[:], in_=xT_d[:])
            wih_sb = pp.tile([128, G4H], bf16)
            nc.sync.dma_start(out=wih_sb[:], in_=wihT_d[:])
            whh_sb = pp.tile([128, 2, G4H], bf16)
            for k in range(2):
                nc.sync.dma_start(out=whh_sb[:, k, :], in_=whhT_d[k])
            biasT_sb = pp.tile([128, NCH], f32)
            nc.sync.dma_start(out=biasT_sb[:], in_=biasT_d[:])
            encT_sb = pp.tile([128, 2, BL, S], bf16)
            for k in range(2):
                nc.sync.dma_start(out=encT_sb[:, k], in_=encT_d[k])
            enc_sb = pp.tile([S, BL, H], bf16)
            nc.sync.dma_start(out=enc_sb[:], in_=enc_d[:])
            cS = pp.tile([128, 2, BL], f32)           # carry, stored as 2*c
            nc.sync.dma_start(out=cS[:], in_=c0T_d[:])
            h0f = pp.tile([128, 2, BL], f32)
            nc.sync.dma_start(out=h0f[:], in_=h0T_d[:])
            h_init = pp.tile([128, 2, BL], bf16)      # 2*h0
            nc.vector.tensor_copy(out=h_init[:], in_=h0f[:])
            lens_i = pp.tile([S, TB, BL], i32)
            lens_bcast = bass.AP(
                tensor=lens_d, offset=0, ap=[[0, S], [0, TB], [1, BL]]
            )
            nc.sync.dma_start(out=lens_i[:], in_=lens_bcast)

            # ---- W_lin^T stream: 64 x [128, 1000], after the small inputs ----
            wlin_sb = pp.tile([128, 4, VL], bf16)
            for g in range(NGRP):
                for k in range(4):
                    nc.sync.dma_start(
                        out=wlin_sb[:, k, g * GC : (g + 1) * GC],
                        in_=wlinT_d[k][:, g * GC : (g + 1) * GC],
                    )

            ident = pp.tile([128, 128], bf16)
            make_identity(nc, ident[:])
            ones_col = pp.tile([S, 1], bf16)
            nc.vector.memset(ones_col[:], 1.0)
            ones_row_f = pp.tile([1, 128], f32)
            nc.vector.memset(ones_row_f[:], 1.0)

            # ---- attention mask, expanded over the 8-step block:
            # mask_e[s, t, b] = 1.0 if s < len_b else 0.0
            lens_f = pp.tile([S, TB, BL], f32)
            nc.vector.tensor_copy(out=lens_f[:], in_=lens_i[:])
            iota_i = pp.tile([S, 1], i32)
            nc.gpsimd.iota(iota_i[:], [[1, 1]], base=0, channel_multiplier=1)
            iota_f = pp.tile([S, 1], f32)
            nc.vector.tensor_copy(out=iota_f[:], in_=iota_i[:])
            mask_e = pp.tile([S, TB, BL], bf16)
            nc.vector.tensor_scalar(
                out=mask_e[:], in0=lens_f[:], scalar1=iota_f[:], scalar2=None,
                op0=mybir.AluOpType.is_gt,
            )

            # ---- persistent state / activations ----
            xwT = pp.tile([128, T, NCH, BL], bf16)    # x@W_ih.T + bias, [p,t,c,b]
            z01 = pp.tile([128, 2, T, BL], bf16)      # 2*h features, (t,b) cols
            z23 = pp.tile([128, 2, T, BL], bf16)      # ctx features, (t,b) cols

            # xw precompute: xwT[p, t, c, b] = (x @ W_ih.T)[tb, c*128+p] + bias
            for c in range(NCH):
                for n0, nn in [(0, 512), (512, 512), (1024, BT - 1024)]:
                    ps_xw = psa.tile([128, 512], f32, tag="att")
                    nc.tensor.matmul(
                        ps_xw[:, :nn],
                        wih_sb[:, c * 128 : (c + 1) * 128],
                        x_allT[:, n0 : n0 + nn],
                        start=True,
                        stop=True,
                    )
                    nc.vector.tensor_scalar(
                        out=xwT[:, n0 // BL : (n0 + nn) // BL, c, :],
                        in0=ps_xw[:, :nn].rearrange("p (t b) -> p t b", b=BL),
                        scalar1=biasT_sb[:, c : c + 1],
                        scalar2=None,
                        op0=mybir.AluOpType.add,
                    )

            # ---- fused pipeline: LSTM steps + per-block attention + GEMM ----
            add = mybir.AluOpType.add
            mult = mybir.AluOpType.mult

            def gemm_group(m, g):
                """Logits m-tile m (rows m*128..), column group g (1000 cols)."""
                ps = psgm.tile([128, 2, 512], f32, tag="po")
                t0 = m * TB
                for k in range(4):
                    zsrc = z01 if k < 2 else z23
                    lhs = zsrc[:, k % 2, t0 : t0 + TB, :].rearrange(
                        "p t b -> p (t b)"
                    )
                    for j in range(2):
                        n0 = g * GC + j * NC
                        nc.tensor.matmul(
                            ps[:, j, :NC],
                            lhs,
                            wlin_sb[:, k, n0 : n0 + NC],
                            start=(k == 0),
                            stop=(k == 3),
                        )
                o_sb = ep.tile([128, 2, NC], bf16, tag="osb")
                if g % 2 == 0:
                    nc.vector.tensor_copy(out=o_sb[:], in_=ps[:, :, :NC])
                else:
                    nc.scalar.copy(out=o_sb[:], in_=ps[:, :, :NC])
                deng = nc.sync if g % 2 == 0 else nc.gpsimd
                deng.dma_start(
                    out=out_d[m * 128 : (m + 1) * 128, g * GC : (g + 1) * GC],
                    in_=o_sb[:].rearrange("p j n -> p (j n)"),
                )

            def attention_a(blk):
                """Scores, exp, mask, denominator (PE stall hides behind the
                recurrence chain of the following step)."""
                t0 = blk * TB
                ps_att = psa.tile([128, 512], f32, tag="att")
                ps_s = ps_att[:S, 0:128]
                for b in range(BL):
                    for k in range(2):
                        nc.tensor.matmul(
                            ps_s[:, b * TB : (b + 1) * TB],
                            encT_sb[:, k, b, :],
                            z01[:, k, t0 : t0 + TB, b],
                            start=(k == 0),
                            stop=(k == 1),
                        )
                expsc = wp.tile([S, BL * TB], bf16, tag="expsc")
                nc.scalar.activation(
                    out=expsc[:], in_=ps_s,
                    func=mybir.ActivationFunctionType.Exp,
                    scale=float(0.5 / np.sqrt(H)),
                )
                nc.vector.tensor_tensor(
                    out=expsc[:].rearrange("s (b t) -> s b t", t=TB),
                    in0=expsc[:].rearrange("s (b t) -> s b t", t=TB),
                    in1=mask_e[:].rearrange("s t b -> s b t"),
                    op=mult,
                )
                nc.tensor.matmul(
                    ps_att[0:1, 128:256], ones_col[:], expsc[:],
                    start=True, stop=True,
                )
                recip = wp.tile([1, 128], f32, tag="recip")
                nc.vector.reciprocal(out=recip[:], in_=ps_att[0:1, 128:256])
                return blk, ps_att, expsc, recip

            def attention_b(blk, ps_att, expsc, recip):
                """Broadcast 1/den, ctx matmuls, z23 write (no PE stalls:
                inputs settled during the previous LSTM step)."""
                t0 = blk * TB
                nc.tensor.matmul(
                    ps_att[:, 128:256], ones_row_f[:], recip[:],
                    start=True, stop=True,
                )
                bc = wp.tile([128, 128], f32, tag="bc")
                nc.vector.tensor_copy(out=bc[:], in_=ps_att[:, 128:256])
                for b in range(BL):
                    for j in range(2):
                        nc.tensor.matmul(
                            ps_att[:, 256 + j * 128 + b * TB :
                                   256 + j * 128 + (b + 1) * TB],
                            enc_sb[:, b, j * 128 : (j + 1) * 128],
                            expsc[:, b * TB : (b + 1) * TB],
                            start=True,
                            stop=True,
                        )
                for j in range(2):
                    nc.vector.tensor_tensor(
                        out=z23[:, j, t0 : t0 + TB, :].rearrange("p t b -> p b t"),
                        in0=ps_att[:, 256 + j * 128 : 256 + (j + 1) * 128]
                        .rearrange("p (b t) -> p b t", t=TB),
                        in1=bc[:].rearrange("p (b t) -> p b t", t=TB),
                        op=mult,
                    )

            pend = None
            for t in range(T):
                # LSTM step t
                ps_g = psg.tile([128, NCH, BL], f32, tag="psg")
                # one accumulation group per step: xw+bias via identity matmul
                # opens it (start), the 16 W_hh matmuls accumulate, last stops
                nc.tensor.matmul(
                    ps_g[:].rearrange("p c b -> p (c b)"),
                    ident[:],
                    xwT[:, t].rearrange("p c b -> p (c b)"),
                    start=True,
                    stop=False,
                )
                for c in range(NCH):
                    for k in range(2):
                        rhs = (
                            h_init[:, k, :] if t == 0
                            else z01[:, k, t - 1, :]
                        )
                        nc.tensor.matmul(
                            ps_g[:, c, :],
                            whh_sb[:, k, c * 128 : (c + 1) * 128],
                            rhs,
                            start=False,
                            stop=(c == NCH - 1 and k == 1),
                        )
                tt = wp.tile([128, NCH, BL], f32, tag="tt")
                nc.scalar.activation(
                    out=tt[:], in_=ps_g[:],
                    func=mybir.ActivationFunctionType.Tanh,
                )
                # chunks 0-1=i, 2-3=f, 4-5=o (all tanh(x/2)), 6-7=g (tanh)
                u = wp.tile([128, 2, BL], f32, tag="u")
                nc.vector.scalar_tensor_tensor(
                    out=u[:], in0=tt[:, 2:4], scalar=1.0, in1=cS[:],
                    op0=add, op1=mult,
                )
                v = wp.tile([128, 2, BL], f32, tag="v")
                nc.vector.scalar_tensor_tensor(
                    out=v[:], in0=tt[:, 0:2], scalar=1.0, in1=tt[:, 6:8],
                    op0=add, op1=mult,
                )
                nc.vector.scalar_tensor_tensor(
                    out=cS[:], in0=u[:], scalar=0.5, in1=v[:],
                    op0=mult, op1=add,
                )
                th = wp.tile([128, 2, BL], f32, tag="th")
                nc.scalar.activation(
                    out=th[:], in_=cS[:],
                    func=mybir.ActivationFunctionType.Tanh,
                    scale=0.5,
                )
                nc.vector.scalar_tensor_tensor(
                    out=z01[:, :, t, :], in0=tt[:, 4:6], scalar=1.0, in1=th[:],
                    op0=add, op1=mult,
                )

                # finish the previous block's attention (ctx) first
                if t % TB == 0 and pend is not None:
                    attention_b(*pend)
                    pend = None

                # interleaved GEMM for the previous block's m-tile
                if t >= TB:
                    m = t // TB - 1
                    g0 = (t % TB) * 2
                    gemm_group(m, g0)
                    gemm_group(m, g0 + 1)

                # attention scores for the block that just finished
                if t % TB == TB - 1:
                    pend = attention_a(t // TB)

            # tail: last block's attention + m-tile GEMM
            attention_b(*pend)
            for g in range(NGRP):
                gemm_group(NBLK - 1, g)

    nc.compile()
    return nc


def _prep_inputs(inputs):
    bf = ml_dtypes.bfloat16
    target = np.asarray(inputs["target_tensor"])
    enc = np.asarray(inputs["encoder_outputs"], dtype=np.float32)
    lens = np.asarray(inputs["encoder_seq_lens"])
    h0 = np.asarray(inputs["h0"], dtype=np.float32)
    c0 = np.asarray(inputs["c0"], dtype=np.float32)
    emb = np.ascontiguousarray(np.asarray(inputs["emb"], dtype=np.float32))
    W_ih = np.asarray(inputs["W_ih"], dtype=np.float32)
    W_hh = np.asarray(inputs["W_hh"], dtype=np.float32)
    bias = (
        np.asarray(inputs["b_ih"], dtype=np.float32)
        + np.asarray(inputs["b_hh"], dtype=np.float32)
    )
    # permute gate order (i, f, g, o) -> (i, f, o, g) so one tanh covers all
    # gates; i/f/o rows are halved so tanh(x/2) gives 2*sigmoid(x)-1
    perm = np.concatenate(
        [np.arange(0, 2 * H), np.arange(3 * H, 4 * H), np.arange(2 * H, 3 * H)]
    )
    W_ih = W_ih[perm]
    W_hh = W_hh[perm]
    bias = bias[perm]
    ifo = 3 * H  # first 768 rows are i, f, o
    W_ih[:ifo] *= 0.5
    bias[:ifo] *= 0.5
    # hidden state is stored as 2h: all W_hh gets an extra 0.5
    W_hh[:ifo] *= 0.25
    W_hh[ifo:] *= 0.5
    W_lin = np.asarray(inputs["W_lin"], dtype=np.float32)
    b_lin = np.asarray(inputs["b_lin"], dtype=np.float32)

    # embedding lookup on host, transposed to [E, (t, b)] per batch group
    x_bt = emb[target.astype(np.int64)].astype(bf)                # (B, T, E)
    wihT = np.ascontiguousarray(W_ih.T.astype(bf))                # (E, 4H)
    whhT = np.ascontiguousarray(
        W_hh.T.reshape(2, 128, G4H).astype(bf)
    )                                                             # (2,128,4H)
    biasT = np.ascontiguousarray(bias.reshape(NCH, 128).T)        # (128, NCH)
    wlinT_full = W_lin.T.copy()                                   # (512, V)
    wlinT_full[:H] *= 0.5                                         # z01 = 2h
    wlinT_full = wlinT_full.astype(bf)

    in_maps = []
    for i in range(NCORES):
        bg = i % NBG
        vh = i // NBG
        sl = slice(bg * BL, (bg + 1) * BL)
        vsl = slice(vh * VL, (vh + 1) * VL)
        xT = np.ascontiguousarray(
            x_bt[sl].transpose(2, 1, 0).reshape(E, BT)
        )  # [E, (t, b)]
        enc_i = enc[sl]                                           # (BL, S, H)
        enc_sbh = np.ascontiguousarray(
            enc_i.transpose(1, 0, 2).astype(bf)
        )                                                         # (S, BL, H)
        encT = np.ascontiguousarray(
            enc_i.transpose(2, 0, 1).reshape(2, 128, BL, S).astype(bf)
        )                                                         # (2,128,BL,S)
        h0T = np.ascontiguousarray(
            (2.0 * h0[sl]).T.reshape(2, 128, BL).transpose(1, 0, 2)
        )
        c0T = np.ascontiguousarray(
            (2.0 * c0[sl]).T.reshape(2, 128, BL).transpose(1, 0, 2)
        )
        wlinT = np.ascontiguousarray(
            wlinT_full[:, vsl].reshape(4, 128, VL)
        )                                                         # (4,128,VL)
        in_maps.append(
            {
                "xT": xT,
                "enc": enc_sbh,
                "encT": encT,
                "h0T": h0T,
                "c0T": c0T,
                "lens": np.ascontiguousarray(lens[sl].astype(np.int32)),
                "biasT": biasT,
                "wihT": wihT,
                "whhT": whhT,
                "wlinT": wlinT,
            }
        )
    return in_maps, b_lin


LAST_RESULTS = None


def _install_ntff_shim():
    """Provide antenv.axon_hooks if the image's antenv lacks it, so
    trace=True/BASS_TRACE=1 can capture NTFF profiles under axon."""
    import sys
    import types

    try:
        from antenv.axon_hooks import get_axon_ntff_profile_hook  # noqa: F401

        return
    except ImportError:
        pass
    try:
        from trn_agent_boot.trn_boot import _ntff_profile_via_ctypes

        hook = _ntff_profile_via_ctypes("/opt/axon/libaxon_pjrt.so")
        m = types.ModuleType("antenv.axon_hooks")
        m.get_axon_ntff_profile_hook = lambda: hook
        m.set_axon_ntff_profile_hook = lambda h: None
        sys.modules["antenv.axon_hooks"] = m
    except Exception:
        pass


def kernel(**inputs):
    global LAST_RESULTS
    _install_ntff_shim()
    if "nc" not in _CACHE:
        _CACHE["nc"] = _build()
    nc = _CACHE["nc"]
    in_maps, b_lin = _prep_inputs(inputs)
    res = run_bass_kernel_spmd(nc, in_maps, core_ids=list(range(NCORES)))
    LAST_RESULTS = res
    out = np.empty((B, T, V), dtype=np.float32)
    for i in range(NCORES):
        bg = i % NBG
        vh = i // NBG
        vsl = slice(vh * VL, (vh + 1) * VL)
        # logits rows are t-major: row = t*BL + b
        lg = res.results[i]["logits"].reshape(T, BL, VL).transpose(1, 0, 2)
        out[bg * BL : (bg + 1) * BL, :, vsl] = lg
        out[bg * BL : (bg + 1) * BL, :, vsl] += b_lin[vsl]
    return out


# revision 42
# speedup vs baseline: 1.1466x; 1.1466x over previous
"""Trainium2 Bass kernel for DecoderAttnRNN (LSTM + attention decoder).

Sharding: hybrid over 8 cores = 4 batch-groups x 2 vocab-halves; each core
runs 16 batches x 16000 vocab columns.

Single fused pipeline in t-major layout (flat row = t*16 + b):
  - LSTM recurrence runs in transposed layout (gate features on partitions,
    batch on the free dim).  All four gates go through ONE tanh activation
    per step: sigma(x) = 0.5*(tanh(x/2)+1), with the 0.5's folded into the
    host-side weight prep (carry is stored as 2c, hidden state as 2h).
    x@W_ih.T + bias is precomputed and added into PSUM with an identity
    matmul, so the per-step chain is PE -> Act -> DVE x3 -> Act -> DVE.
  - Attention for each 8-step block runs right after the block's last step.
  - The 512x16000 logits GEMM for block m is interleaved into block m+1's
    LSTM steps (2 x 1000-col groups per step) so the PE array streams
    continuously instead of idling during the recurrence.
  - Logits are written to DRAM as bf16 (halves output traffic); b_lin is
    added on the host during unsharding.
"""

import numpy as np
import ml_dtypes

import concourse.bass as bass
import concourse.mybir as mybir
import concourse.tile as tile
from concourse import bacc
from concourse.bass_utils import run_bass_kernel_spmd
from concourse.masks import make_identity

B, T, S, E, H, V = 64, 72, 72, 128, 256, 32000
NCORES = 8
NBG = 4                   # batch groups
NVH = 2                   # vocab halves
BL = B // NBG             # 16 batches per core
BT = BL * T               # 1152 (t-major: flat index = t*BL + b)
VL = V // NVH             # 16000 vocab cols per core
G4H = 4 * H               # 1024
NCH = G4H // 128          # 8 gate chunks of 128
TB = 8                    # timesteps per block
NBLK = T // TB            # 9 blocks == 9 GEMM m-tiles
NGRP = 16                 # GEMM column groups per m-tile (1000 cols each)
GC = VL // NGRP           # 1000
NC = GC // 2              # 500 cols per PSUM-bank chunk

f32 = mybir.dt.float32
bf16 = mybir.dt.bfloat16
i32 = mybir.dt.int32

_CACHE = {}


def _build():
    nc = bacc.Bacc(None, target_bir_lowering=False)

    xT_d = nc.declare_dram_parameter("xT", [E, BT], bf16, isOutput=False)
    enc_d = nc.declare_dram_parameter("enc", [S, BL, H], bf16, isOutput=False)
    encT_d = nc.declare_dram_parameter("encT", [2, 128, BL, S], bf16, isOutput=False)
    h0T_d = nc.declare_dram_parameter("h0T", [128, 2, BL], f32, isOutput=False)
    c0T_d = nc.declare_dram_parameter("c0T", [128, 2, BL], f32, isOutput=False)
    lens_d = nc.declare_dram_parameter("lens", [BL], i32, isOutput=False)
    biasT_d = nc.declare_dram_parameter("biasT", [128, NCH], f32, isOutput=False)
    wihT_d = nc.declare_dram_parameter("wihT", [E, G4H], bf16, isOutput=False)
    whhT_d = nc.declare_dram_parameter("whhT", [2, 128, G4H], bf16, isOutput=False)
    wlinT_d = nc.declare_dram_parameter("wlinT", [4, 128, VL], bf16, isOutput=False)
    out_d = nc.declare_dram_parameter("logits", [BT, VL], bf16, isOutput=True)

    with tile.TileContext(nc) as tc:
        with (
            tc.tile_pool(name="persist", bufs=1) as pp,
            tc.tile_pool(name="work", bufs=2) as wp,
            tc.tile_pool(name="evict", bufs=4) as ep,
            tc.tile_pool(name="psg", bufs=2, space="PSUM") as psg,
            tc.tile_pool(name="psa", bufs=1, space="PSUM") as psa,
            tc.tile_pool(name="psgm", bufs=2, space="PSUM") as psgm,
        ):
            # ---- small persistent inputs, all on the sync queue BEFORE the
            # W_lin stream so they are not starved behind 16 MB of weights ----
            x_allT = pp.tile([128, BT], bf16)         # embeddings^T, (t,b) cols
            nc.sync.dma_start(out=x_allT[:], in_=xT_d[:])
            wih_sb = pp.tile([128, G4H], bf16)
            nc.sync.dma_start(out=wih_sb[:], in_=wihT_d[:])
            whh_sb = pp.tile([128, 2, G4H], bf16)
            for k in range(2):
                nc.sync.dma_start(out=whh_sb[:, k, :], in_=whhT_d[k])
            biasT_sb = pp.tile([128, NCH], f32)
            nc.sync.dma_start(out=biasT_sb[:], in_=biasT_d[:])
            encT_sb = pp.tile([128, 2, BL, S], bf16)
            for k in range(2):
                nc.sync.dma_start(out=encT_sb[:, k], in_=encT_d[k])
            enc_sb = pp.tile([S, BL, H], bf16)
            nc.sync.dma_start(out=enc_sb[:], in_=enc_d[:])
            cS = pp.tile([128, 2, BL], f32)           # carry, stored as 2*c
            nc.sync.dma_start(out=cS[:], in_=c0T_d[:])
            h0f = pp.tile([128, 2, BL], f32)
            nc.sync.dma_start(out=h0f[:], in_=h0T_d[:])
            h_init = pp.tile([128, 2, BL], bf16)      # 2*h0
            nc.vector.tensor_copy(out=h_init[:], in_=h0f[:])
            lens_i = pp.tile([S, TB, BL], i32)
            lens_bcast = bass.AP(
                tensor=lens_d, offset=0, ap=[[0, S], [0, TB], [1, BL]]
            )
            nc.sync.dma_start(out=lens_i[:], in_=lens_bcast)

            # ---- W_lin^T stream: 64 x [128, 1000], after the small inputs ----
            wlin_sb = pp.tile([128, 4, VL], bf16)
            for g in range(NGRP):
                for k in range(4):
                    nc.sync.dma_start(
                        out=wlin_sb[:, k, g * GC : (g + 1) * GC],
                        in_=wlinT_d[k][:, g * GC : (g + 1) * GC],
                    )

            ident = pp.tile([128, 128], bf16)
            make_identity(nc, ident[:])
            ones_col = pp.tile([S, 1], bf16)
            nc.vector.memset(ones_col[:], 1.0)
            ones_row_f = pp.tile([1, 128], f32)
            nc.vector.memset(ones_row_f[:], 1.0)

            # ---- attention mask, expanded over the 8-step block:
            # mask_e[s, t, b] = 1.0 if s < len_b else 0.0
            lens_f = pp.tile([S, TB, BL], f32)
            nc.vector.tensor_copy(out=lens_f[:], in_=lens_i[:])
            iota_i = pp.tile([S, 1], i32)
            nc.gpsimd.iota(iota_i[:], [[1, 1]], base=0, channel_multiplier=1)
            iota_f = pp.tile([S, 1], f32)
            nc.vector.tensor_copy(out=iota_f[:], in_=iota_i[:])
            mask_e = pp.tile([S, TB, BL], bf16)
            nc.vector.tensor_scalar(
                out=mask_e[:], in0=lens_f[:], scalar1=iota_f[:], scalar2=None,
                op0=mybir.AluOpType.is_gt,
            )

            # ---- persistent state / activations ----
            xwT = pp.tile([128, T, NCH, BL], bf16)    # x@W_ih.T + bias, [p,t,c,b]
            z01 = pp.tile([128, 2, T, BL], bf16)      # 2*h features, (t,b) cols
            z23 = pp.tile([128, 2, T, BL], bf16)      # ctx features, (t,b) cols

            # xw precompute: xwT[p, t, c, b] = (x @ W_ih.T)[tb, c*128+p] + bias
            for c in range(NCH):
                for n0, nn in [(0, 512), (512, 512), (1024, BT - 1024)]:
                    ps_xw = psa.tile([128, 512], f32, tag="att")
                    nc.tensor.matmul(
                        ps_xw[:, :nn],
                        wih_sb[:, c * 128 : (c + 1) * 128],
                        x_allT[:, n0 : n0 + nn],
                        start=True,
                        stop=True,
                    )
                    nc.vector.tensor_scalar(
                        out=xwT[:, n0 // BL : (n0 + nn) // BL, c, :],
                        in0=ps_xw[:, :nn].rearrange("p (t b) -> p t b", b=BL),
                        scalar1=biasT_sb[:, c : c + 1],
                        scalar2=None,
                        op0=mybir.AluOpType.add,
                    )

            # ---- fused pipeline: LSTM steps + per-block attention + GEMM ----
            add = mybir.AluOpType.add
            mult = mybir.AluOpType.mult

            def gemm_group(m, g):
                """Logits m-tile m (rows m*128..), column group g (1000 cols)."""
                ps = psgm.tile([128, 2, 512], f32, tag="po")
                t0 = m * TB
                for k in range(4):
                    zsrc = z01 if k < 2 else z23
                    lhs = zsrc[:, k % 2, t0 : t0 + TB, :].rearrange(
                        "p t b -> p (t b)"
                    )
                    for j in range(2):
                        n0 = g * GC + j * NC
                        nc.tensor.matmul(
                            ps[:, j, :NC],
                            lhs,
                            wlin_sb[:, k, n0 : n0 + NC],
                            start=(k == 0),
                            stop=(k == 3),
                        )
                o_sb = ep.tile([128, 2, NC], bf16, tag="osb")
                if g % 2 == 0:
                    nc.vector.tensor_copy(out=o_sb[:], in_=ps[:, :, :NC])
                else:
                    nc.scalar.copy(out=o_sb[:], in_=ps[:, :, :NC])
                deng = nc.sync if g % 2 == 0 else nc.gpsimd
                deng.dma_start(
                    out=out_d[m * 128 : (m + 1) * 128, g * GC : (g + 1) * GC],
                    in_=o_sb[:].rearrange("p j n -> p (j n)"),
                )

            def attention_a(blk):
                """Scores, exp, mask, denominator (PE stall hides behind the
                recurrence chain of the following step)."""
                t0 = blk * TB
                ps_att = psa.tile([128, 512], f32, tag="att")
                ps_s = ps_att[:S, 0:128]
                for b in range(BL):
                    for k in range(2):
                        nc.tensor.matmul(
                            ps_s[:, b * TB : (b + 1) * TB],
                            encT_sb[:, k, b, :],
                            z01[:, k, t0 : t0 + TB, b],
                            start=(k == 0),
                            stop=(k == 1),
                        )
                expsc = wp.tile([S, BL * TB], bf16, tag="expsc")
                nc.scalar.activation(
                    out=expsc[:], in_=ps_s,
                    func=mybir.ActivationFunctionType.Exp,
                    scale=float(0.5 / np.sqrt(H)),
                )
                nc.vector.tensor_tensor(
                    out=expsc[:].rearrange("s (b t) -> s b t", t=TB),
                    in0=expsc[:].rearrange("s (b t) -> s b t", t=TB),
                    in1=mask_e[:].rearrange("s t b -> s b t"),
                    op=mult,
                )
                nc.tensor.matmul(
                    ps_att[0:1, 128:256], ones_col[:], expsc[:],
                    start=True, stop=True,
                )
                recip = wp.tile([1, 128], f32, tag="recip")
                nc.vector.reciprocal(out=recip[:], in_=ps_att[0:1, 128:256])
                return blk, ps_att, expsc, recip

            def attention_b(blk, ps_att, expsc, recip):
                """Broadcast 1/den, ctx matmuls, z23 write (no PE stalls:
                inputs settled during the previous LSTM step)."""
                t0 = blk * TB
                nc.tensor.matmul(
                    ps_att[:, 128:256], ones_row_f[:], recip[:],
                    start=True, stop=True,
                )
                bc = wp.tile([128, 128], f32, tag="bc")
                nc.vector.tensor_copy(out=bc[:], in_=ps_att[:, 128:256])
                for b in range(BL):
                    for j in range(2):
                        nc.tensor.matmul(
                            ps_att[:, 256 + j * 128 + b * TB :
                                   256 + j * 128 + (b + 1) * TB],
                            enc_sb[:, b, j * 128 : (j + 1) * 128],
                            expsc[:, b * TB : (b + 1) * TB],
                            start=True,
                            stop=True,
                        )
                for j in range(2):
                    nc.vector.tensor_tensor(
                        out=z23[:, j, t0 : t0 + TB, :].rearrange("p t b -> p b t"),
                        in0=ps_att[:, 256 + j * 128 : 256 + (j + 1) * 128]
                        .rearrange("p (b t) -> p b t", t=TB),
                        in1=bc[:].rearrange("p (b t) -> p b t", t=TB),
                        op=mult,
                    )

            pend = None
            for t in range(T):
                # LSTM step t
                ps_g = psg.tile([128, NCH, BL], f32, tag="psg")
                # one accumulation group per step: xw+bias via identity matmul
                # opens it (start), the 16 W_hh matmuls accumulate, last stops
                nc.tensor.matmul(
                    ps_g[:].rearrange("p c b -> p (c b)"),
                    ident[:],
                    xwT[:, t].rearrange("p c b -> p (c b)"),
                    start=True,
                    stop=False,
                )
                for c in range(NCH):
                    for k in range(2):
                        rhs = (
                            h_init[:, k, :] if t == 0
                            else z01[:, k, t - 1, :]
                        )
                        nc.tensor.matmul(
                            ps_g[:, c, :],
                            whh_sb[:, k, c * 128 : (c + 1) * 128],
                            rhs,
                            start=False,
                            stop=(c == NCH - 1 and k == 1),
                        )
                tt = wp.tile([128, NCH, BL], f32, tag="tt")
                nc.scalar.activation(
                    out=tt[:], in_=ps_g[:],
                    func=mybir.ActivationFunctionType.Tanh,
                )
                # chunks 0-1=i, 2-3=f, 4-5=o (all tanh(x/2)), 6-7=g (tanh)
                u = wp.tile([128, 2, BL], f32, tag="u")
                nc.vector.scalar_tensor_tensor(
                    out=u[:], in0=tt[:, 2:4], scalar=1.0, in1=cS[:],
                    op0=add, op1=mult,
                )
                v = wp.tile([128, 2, BL], f32, tag="v")
                nc.vector.scalar_tensor_tensor(
                    out=v[:], in0=tt[:, 0:2], scalar=1.0, in1=tt[:, 6:8],
                    op0=add, op1=mult,
                )
                nc.vector.scalar_tensor_tensor(
                    out=cS[:], in0=u[:], scalar=0.5, in1=v[:],
                    op0=mult, op1=add,
                )
                th = wp.tile([128, 2, BL], f32, tag="th")
                nc.scalar.activation(
                    out=th[:], in_=cS[:],
                    func=mybir.ActivationFunctionType.Tanh,
                    scale=0.5,
                )
                nc.vector.scalar_tensor_tensor(
                    out=z01[:, :, t, :], in0=tt[:, 4:6], scalar=1.0, in1=th[:],
                    op0=add, op1=mult,
                )

                # finish the previous block's attention (ctx) first
                if t % TB == 0 and pend is not None:
                    attention_b(*pend)
                    pend = None

                # interleaved GEMM for the previous block's m-tile
                if t >= TB:
                    m = t // TB - 1
                    g0 = (t % TB) * 2
                    gemm_group(m, g0)
                    gemm_group(m, g0 + 1)

                # attention scores for the block that just finished
                if t % TB == TB - 1:
                    pend = attention_a(t // TB)

            # tail: last block's attention + m-tile GEMM
            attention_b(*pend)
            for g in range(NGRP):
                gemm_group(NBLK - 1, g)

    nc.compile()
    return nc


def _prep_inputs(inputs):
    bf = ml_dtypes.bfloat16
    target = np.asarray(inputs["target_tensor"])
    enc = np.asarray(inputs["encoder_outputs"], dtype=np.float32)
    lens = np.asarray(inputs["encoder_seq_lens"])
    h0 = np.asarray(inputs["h0"], dtype=np.float32)
    c0 = np.asarray(inputs["c0"], dtype=np.float32)
    emb = np.ascontiguousarray(np.asarray(inputs["emb"], dtype=np.float32))
    W_ih = np.asarray(inputs["W_ih"], dtype=np.float32)
    W_hh = np.asarray(inputs["W_hh"], dtype=np.float32)
    bias = (
        np.asarray(inputs["b_ih"], dtype=np.float32)
        + np.asarray(inputs["b_hh"], dtype=np.float32)
    )
    # permute gate order (i, f, g, o) -> (i, f, o, g) so one tanh covers all
    # gates; i/f/o rows are halved so tanh(x/2) gives 2*sigmoid(x)-1
    perm = np.concatenate(
        [np.arange(0, 2 * H), np.arange(3 * H, 4 * H), np.arange(2 * H, 3 * H)]
    )
    W_ih = W_ih[perm]
    W_hh = W_hh[perm]
    bias = bias[perm]
    ifo = 3 * H  # first 768 rows are i, f, o
    W_ih[:ifo] *= 0.5
    bias[:ifo] *= 0.5
    # hidden state is stored as 2h: all W_hh gets an extra 0.5
    W_hh[:ifo] *= 0.25
    W_hh[ifo:] *= 0.5
    W_lin = np.asarray(inputs["W_lin"], dtype=np.float32)
    b_lin = np.asarray(inputs["b_lin"], dtype=np.float32)

    # embedding lookup on host, transposed to [E, (t, b)] per batch group
    x_bt = emb[target.astype(np.int64)].astype(bf)                # (B, T, E)
    wihT = np.ascontiguousarray(W_ih.T.astype(bf))                # (E, 4H)
    whhT = np.ascontiguousarray(
        W_hh.T.reshape(2, 128, G4H).astype(bf)
    )                                                             # (2,128,4H)
    biasT = np.ascontiguousarray(bias.reshape(NCH, 128).T)        # (128, NCH)
    wlinT_full = W_lin.T.copy()                                   # (512, V)
    wlinT_full[:H] *= 0.5                                         # z01 = 2h
    wlinT_full = wlinT_full.astype(bf)

    in_maps = []
    for i in range(NCORES):
        bg = i % NBG
        vh = i // NBG
        sl = slice(bg * BL, (bg + 1) * BL)
        vsl = slice(vh * VL, (vh + 1) * VL)
        xT = np.ascontiguousarray(
            x_bt[sl].transpose(2, 1, 0).reshape(E, BT)
        )  # [E, (t, b)]
        enc_i = enc[sl]                                           # (BL, S, H)
        enc_sbh = np.ascontiguousarray(
            enc_i.transpose(1, 0, 2).astype(bf)
        )                                                         # (S, BL, H)
        encT = np.ascontiguousarray(
            enc_i.transpose(2, 0, 1).reshape(2, 128, BL, S).astype(bf)
        )                                                         # (2,128,BL,S)
        h0T = np.ascontiguousarray(
            (2.0 * h0[sl]).T.reshape(2, 128, BL).transpose(1, 0, 2)
        )
        c0T = np.ascontiguousarray(
            (2.0 * c0[sl]).T.reshape(2, 128, BL).transpose(1, 0, 2)
        )
        wlinT = np.ascontiguousarray(
            wlinT_full[:, vsl].reshape(4, 128, VL)
        )                                                         # (4,128,VL)
        in_maps.append(
            {
                "xT": xT,
                "enc": enc_sbh,
                "encT": encT,
                "h0T": h0T,
                "c0T": c0T,
                "lens": np.ascontiguousarray(lens[sl].astype(np.int32)),
                "biasT": biasT,
                "wihT": wihT,
                "whhT": whhT,
                "wlinT": wlinT,
            }
        )
    return in_maps, b_lin


LAST_RESULTS = None


def _install_ntff_shim():
    """Provide antenv.axon_hooks if the image's antenv lacks it, so
    trace=True/BASS_TRACE=1 can capture NTFF profiles under axon."""
    import sys
    import types

    try:
        from antenv.axon_hooks import get_axon_ntff_profile_hook  # noqa: F401

        return
    except ImportError:
        pass
    try:
        from trn_agent_boot.trn_boot import _ntff_profile_via_ctypes

        hook = _ntff_profile_via_ctypes("/opt/axon/libaxon_pjrt.so")
        m = types.ModuleType("antenv.axon_hooks")
        m.get_axon_ntff_profile_hook = lambda: hook
        m.set_axon_ntff_profile_hook = lambda h: None
        sys.modules["antenv.axon_hooks"] = m
    except Exception:
        pass


def kernel(**inputs):
    global LAST_RESULTS
    _install_ntff_shim()
    if "nc" not in _CACHE:
        _CACHE["nc"] = _build()
    nc = _CACHE["nc"]
    in_maps, b_lin = _prep_inputs(inputs)
    res = run_bass_kernel_spmd(nc, in_maps, core_ids=list(range(NCORES)))
    LAST_RESULTS = res
    out = np.empty((B, T, V), dtype=np.float32)
    for i in range(NCORES):
        bg = i % NBG
        vh = i // NBG
        vsl = slice(vh * VL, (vh + 1) * VL)
        # logits rows are t-major: row = t*BL + b
        lg = res.results[i]["logits"].reshape(T, BL, VL).transpose(1, 0, 2)
        out[bg * BL : (bg + 1) * BL, :, vsl] = lg
        out[bg * BL : (bg + 1) * BL, :, vsl] += b_lin[vsl]
    return out


# revision 43
# speedup vs baseline: 1.1574x; 1.0094x over previous
"""Trainium2 Bass kernel for DecoderAttnRNN (LSTM + attention decoder).

Sharding: hybrid over 8 cores = 4 batch-groups x 2 vocab-halves; each core
runs 16 batches x 16000 vocab columns.

Single fused pipeline in t-major layout (flat row = t*16 + b):
  - LSTM recurrence runs in transposed layout (gate features on partitions,
    batch on the free dim).  All four gates go through ONE tanh activation
    per step: sigma(x) = 0.5*(tanh(x/2)+1), with the 0.5's folded into the
    host-side weight prep (carry is stored as 2c, hidden state as 2h).
    x@W_ih.T + bias is precomputed and added into PSUM with an identity
    matmul, so the per-step chain is PE -> Act -> DVE x3 -> Act -> DVE.
  - Attention for each 8-step block runs right after the block's last step.
  - The 512x16000 logits GEMM for block m is interleaved into block m+1's
    LSTM steps (2 x 1000-col groups per step) so the PE array streams
    continuously instead of idling during the recurrence.
  - Logits are written to DRAM as bf16 (halves output traffic); b_lin is
    added on the host during unsharding.
"""

import numpy as np
import ml_dtypes

import concourse.bass as bass
import concourse.mybir as mybir
import concourse.tile as tile
from concourse import bacc
from concourse.bass_utils import run_bass_kernel_spmd
from concourse.masks import make_identity

B, T, S, E, H, V = 64, 72, 72, 128, 256, 32000
NCORES = 8
NBG = 4                   # batch groups
NVH = 2                   # vocab halves
BL = B // NBG             # 16 batches per core
BT = BL * T               # 1152 (t-major: flat index = t*BL + b)
VL = V // NVH             # 16000 vocab cols per core
G4H = 4 * H               # 1024
NCH = G4H // 128          # 8 gate chunks of 128
TB = 8                    # timesteps per block
NBLK = T // TB            # 9 blocks == 9 GEMM m-tiles
NGRP = 16                 # GEMM column groups per m-tile (1000 cols each)
GC = VL // NGRP           # 1000
NC = GC // 2              # 500 cols per PSUM-bank chunk

f32 = mybir.dt.float32
bf16 = mybir.dt.bfloat16
i32 = mybir.dt.int32

_CACHE = {}


def _build():
    nc = bacc.Bacc(None, target_bir_lowering=False)

    xT_d = nc.declare_dram_parameter("xT", [E, BT], bf16, isOutput=False)
    enc_d = nc.declare_dram_parameter("enc", [S, BL, H], bf16, isOutput=False)
    encT_d = nc.declare_dram_parameter("encT", [2, 128, BL, S], bf16, isOutput=False)
    h0T_d = nc.declare_dram_parameter("h0T", [128, 2, BL], f32, isOutput=False)
    c0T_d = nc.declare_dram_parameter("c0T", [128, 2, BL], f32, isOutput=False)
    lens_d = nc.declare_dram_parameter("lens", [BL], i32, isOutput=False)
    biasT_d = nc.declare_dram_parameter("biasT", [128, NCH], f32, isOutput=False)
    wihT_d = nc.declare_dram_parameter("wihT", [E, G4H], bf16, isOutput=False)
    whhT_d = nc.declare_dram_parameter("whhT", [2, 128, G4H], bf16, isOutput=False)
    wlinT_d = nc.declare_dram_parameter("wlinT", [4, 128, VL], bf16, isOutput=False)
    out_d = nc.declare_dram_parameter("logits", [BT, VL], bf16, isOutput=True)

    with tile.TileContext(nc) as tc:
        with (
            tc.tile_pool(name="persist", bufs=1) as pp,
            tc.tile_pool(name="work", bufs=2) as wp,
            tc.tile_pool(name="evict", bufs=4) as ep,
            tc.tile_pool(name="psg", bufs=2, space="PSUM") as psg,
            tc.tile_pool(name="psa", bufs=2, space="PSUM") as psa,
            tc.tile_pool(name="psgm", bufs=2, space="PSUM") as psgm,
        ):
            # ---- small persistent inputs, all on the sync queue BEFORE the
            # W_lin stream so they are not starved behind 16 MB of weights ----
            x_allT = pp.tile([128, BT], bf16)         # embeddings^T, (t,b) cols
            nc.sync.dma_start(out=x_allT[:], in_=xT_d[:])
            wih_sb = pp.tile([128, G4H], bf16)
            nc.sync.dma_start(out=wih_sb[:], in_=wihT_d[:])
            whh_sb = pp.tile([128, 2, G4H], bf16)
            for k in range(2):
                nc.sync.dma_start(out=whh_sb[:, k, :], in_=whhT_d[k])
            biasT_sb = pp.tile([128, NCH], f32)
            nc.sync.dma_start(out=biasT_sb[:], in_=biasT_d[:])
            encT_sb = pp.tile([128, 2, BL, S], bf16)
            for k in range(2):
                nc.sync.dma_start(out=encT_sb[:, k], in_=encT_d[k])
            enc_sb = pp.tile([S, BL, H], bf16)
            nc.sync.dma_start(out=enc_sb[:], in_=enc_d[:])
            cS = pp.tile([128, 2, BL], f32)           # carry, stored as 2*c
            nc.sync.dma_start(out=cS[:], in_=c0T_d[:])
            h0f = pp.tile([128, 2, BL], f32)
            nc.sync.dma_start(out=h0f[:], in_=h0T_d[:])
            h_init = pp.tile([128, 2, BL], bf16)      # 2*h0
            nc.vector.tensor_copy(out=h_init[:], in_=h0f[:])
            lens_i = pp.tile([S, TB, BL], i32)
            lens_bcast = bass.AP(
                tensor=lens_d, offset=0, ap=[[0, S], [0, TB], [1, BL]]
            )
            nc.sync.dma_start(out=lens_i[:], in_=lens_bcast)

            # ---- W_lin^T stream: 64 x [128, 1000], after the small inputs ----
            wlin_sb = pp.tile([128, 4, VL], bf16)
            for g in range(NGRP):
                for k in range(4):
                    nc.sync.dma_start(
                        out=wlin_sb[:, k, g * GC : (g + 1) * GC],
                        in_=wlinT_d[k][:, g * GC : (g + 1) * GC],
                    )

            ident = pp.tile([128, 128], bf16)
            make_identity(nc, ident[:])
            ones_col = pp.tile([S, 1], bf16)
            nc.vector.memset(ones_col[:], 1.0)
            ones_row_f = pp.tile([1, 128], f32)
            nc.vector.memset(ones_row_f[:], 1.0)

            # ---- attention mask, expanded over the 8-step block:
            # mask_e[s, t, b] = 1.0 if s < len_b else 0.0
            lens_f = pp.tile([S, TB, BL], f32)
            nc.vector.tensor_copy(out=lens_f[:], in_=lens_i[:])
            iota_i = pp.tile([S, 1], i32)
            nc.gpsimd.iota(iota_i[:], [[1, 1]], base=0, channel_multiplier=1)
            iota_f = pp.tile([S, 1], f32)
            nc.vector.tensor_copy(out=iota_f[:], in_=iota_i[:])
            mask_e = pp.tile([S, TB, BL], bf16)
            nc.vector.tensor_scalar(
                out=mask_e[:], in0=lens_f[:], scalar1=iota_f[:], scalar2=None,
                op0=mybir.AluOpType.is_gt,
            )

            # ---- persistent state / activations ----
            xwT = pp.tile([128, T, NCH, BL], bf16)    # x@W_ih.T + bias, [p,t,c,b]
            z01 = pp.tile([128, 2, T, BL], bf16)      # 2*h features, (t,b) cols
            z23 = pp.tile([128, 2, T, BL], bf16)      # ctx features, (t,b) cols

            # xw precompute: xwT[p, t, c, b] = (x @ W_ih.T)[tb, c*128+p] + bias
            for c in range(NCH):
                for n0, nn in [(0, 512), (512, 512), (1024, BT - 1024)]:
                    ps_xw = psa.tile([128, 512], f32, tag="att")
                    nc.tensor.matmul(
                        ps_xw[:, :nn],
                        wih_sb[:, c * 128 : (c + 1) * 128],
                        x_allT[:, n0 : n0 + nn],
                        start=True,
                        stop=True,
                    )
                    nc.vector.tensor_scalar(
                        out=xwT[:, n0 // BL : (n0 + nn) // BL, c, :],
                        in0=ps_xw[:, :nn].rearrange("p (t b) -> p t b", b=BL),
                        scalar1=biasT_sb[:, c : c + 1],
                        scalar2=None,
                        op0=mybir.AluOpType.add,
                    )

            # ---- fused pipeline: LSTM steps + per-block attention + GEMM ----
            add = mybir.AluOpType.add
            mult = mybir.AluOpType.mult

            def gemm_group(m, g):
                """Logits m-tile m (rows m*128..), column group g (1000 cols)."""
                ps = psgm.tile([128, 2, 512], f32, tag="po")
                t0 = m * TB
                for k in range(4):
                    zsrc = z01 if k < 2 else z23
                    lhs = zsrc[:, k % 2, t0 : t0 + TB, :].rearrange(
                        "p t b -> p (t b)"
                    )
                    for j in range(2):
                        n0 = g * GC + j * NC
                        nc.tensor.matmul(
                            ps[:, j, :NC],
                            lhs,
                            wlin_sb[:, k, n0 : n0 + NC],
                            start=(k == 0),
                            stop=(k == 3),
                        )
                o_sb = ep.tile([128, 2, NC], bf16, tag="osb")
                if g % 2 == 0:
                    nc.vector.tensor_copy(out=o_sb[:], in_=ps[:, :, :NC])
                else:
                    nc.scalar.copy(out=o_sb[:], in_=ps[:, :, :NC])
                deng = nc.sync if g % 2 == 0 else nc.gpsimd
                deng.dma_start(
                    out=out_d[m * 128 : (m + 1) * 128, g * GC : (g + 1) * GC],
                    in_=o_sb[:].rearrange("p j n -> p (j n)"),
                )

            def attention_a(blk):
                """Scores, exp, mask, denominator (PE stall hides behind the
                recurrence chain of the following step)."""
                t0 = blk * TB
                ps_att = psa.tile([128, 512], f32, tag="att")
                ps_s = ps_att[:S, 0:128]
                for b in range(BL):
                    for k in range(2):
                        nc.tensor.matmul(
                            ps_s[:, b * TB : (b + 1) * TB],
                            encT_sb[:, k, b, :],
                            z01[:, k, t0 : t0 + TB, b],
                            start=(k == 0),
                            stop=(k == 1),
                        )
                expsc = wp.tile([S, BL * TB], bf16, tag="expsc")
                nc.scalar.activation(
                    out=expsc[:], in_=ps_s,
                    func=mybir.ActivationFunctionType.Exp,
                    scale=float(0.5 / np.sqrt(H)),
                )
                nc.vector.tensor_tensor(
                    out=expsc[:].rearrange("s (b t) -> s b t", t=TB),
                    in0=expsc[:].rearrange("s (b t) -> s b t", t=TB),
                    in1=mask_e[:].rearrange("s t b -> s b t"),
                    op=mult,
                )
                nc.tensor.matmul(
                    ps_att[0:1, 128:256], ones_col[:], expsc[:],
                    start=True, stop=True,
                )
                recip = wp.tile([1, 128], f32, tag="recip")
                nc.vector.reciprocal(out=recip[:], in_=ps_att[0:1, 128:256])
                return blk, ps_att, expsc, recip

            def attention_b(blk, ps_att, expsc, recip):
                """Broadcast 1/den, ctx matmuls, z23 write (no PE stalls:
                inputs settled during the previous LSTM step)."""
                t0 = blk * TB
                nc.tensor.matmul(
                    ps_att[:, 128:256], ones_row_f[:], recip[:],
                    start=True, stop=True,
                )
                bc = wp.tile([128, 128], f32, tag="bc")
                nc.vector.tensor_copy(out=bc[:], in_=ps_att[:, 128:256])
                for b in range(BL):
                    for j in range(2):
                        nc.tensor.matmul(
                            ps_att[:, 256 + j * 128 + b * TB :
                                   256 + j * 128 + (b + 1) * TB],
                            enc_sb[:, b, j * 128 : (j + 1) * 128],
                            expsc[:, b * TB : (b + 1) * TB],
                            start=True,
                            stop=True,
                        )
                for j in range(2):
                    nc.vector.tensor_tensor(
                        out=z23[:, j, t0 : t0 + TB, :].rearrange("p t b -> p b t"),
                        in0=ps_att[:, 256 + j * 128 : 256 + (j + 1) * 128]
                        .rearrange("p (b t) -> p b t", t=TB),
                        in1=bc[:].rearrange("p (b t) -> p b t", t=TB),
                        op=mult,
                    )

            pend = None
            for t in range(T):
                # LSTM step t
                ps_g = psg.tile([128, NCH, BL], f32, tag="psg")
                # one accumulation group per step: xw+bias via identity matmul
                # opens it (start), the 16 W_hh matmuls accumulate, last stops
                nc.tensor.matmul(
                    ps_g[:].rearrange("p c b -> p (c b)"),
                    ident[:],
                    xwT[:, t].rearrange("p c b -> p (c b)"),
                    start=True,
                    stop=False,
                )
                for c in range(NCH):
                    for k in range(2):
                        rhs = (
                            h_init[:, k, :] if t == 0
                            else z01[:, k, t - 1, :]
                        )
                        nc.tensor.matmul(
                            ps_g[:, c, :],
                            whh_sb[:, k, c * 128 : (c + 1) * 128],
                            rhs,
                            start=False,
                            stop=(c == NCH - 1 and k == 1),
                        )
                tt = wp.tile([128, NCH, BL], f32, tag="tt")
                nc.scalar.activation(
                    out=tt[:], in_=ps_g[:],
                    func=mybir.ActivationFunctionType.Tanh,
                )
                # chunks 0-1=i, 2-3=f, 4-5=o (all tanh(x/2)), 6-7=g (tanh)
                u = wp.tile([128, 2, BL], f32, tag="u")
                nc.vector.scalar_tensor_tensor(
                    out=u[:], in0=tt[:, 2:4], scalar=1.0, in1=cS[:],
                    op0=add, op1=mult,
                )
                v = wp.tile([128, 2, BL], f32, tag="v")
                nc.vector.scalar_tensor_tensor(
                    out=v[:], in0=tt[:, 0:2], scalar=1.0, in1=tt[:, 6:8],
                    op0=add, op1=mult,
                )
                nc.vector.scalar_tensor_tensor(
                    out=cS[:], in0=u[:], scalar=0.5, in1=v[:],
                    op0=mult, op1=add,
                )
                th = wp.tile([128, 2, BL], f32, tag="th")
                nc.scalar.activation(
                    out=th[:], in_=cS[:],
                    func=mybir.ActivationFunctionType.Tanh,
                    scale=0.5,
                )
                nc.vector.scalar_tensor_tensor(
                    out=z01[:, :, t, :], in0=tt[:, 4:6], scalar=1.0, in1=th[:],
                    op0=add, op1=mult,
                )

                # finish the previous block's attention (ctx) first
                if t % TB == 0 and pend is not None:
                    attention_b(*pend)
                    pend = None

                # interleaved GEMM for the previous block's m-tile
                if t >= TB:
                    m = t // TB - 1
                    g0 = (t % TB) * 2
                    gemm_group(m, g0)
                    gemm_group(m, g0 + 1)

                # attention scores for the block that just finished
                if t % TB == TB - 1:
                    pend = attention_a(t // TB)

            # tail: last block's attention + m-tile GEMM
            attention_b(*pend)
            for g in range(NGRP):
                gemm_group(NBLK - 1, g)

    nc.compile()
    return nc


def _prep_inputs(inputs):
    bf = ml_dtypes.bfloat16
    target = np.asarray(inputs["target_tensor"])
    enc = np.asarray(inputs["encoder_outputs"], dtype=np.float32)
    lens = np.asarray(inputs["encoder_seq_lens"])
    h0 = np.asarray(inputs["h0"], dtype=np.float32)
    c0 = np.asarray(inputs["c0"], dtype=np.float32)
    emb = np.ascontiguousarray(np.asarray(inputs["emb"], dtype=np.float32))
    W_ih = np.asarray(inputs["W_ih"], dtype=np.float32)
    W_hh = np.asarray(inputs["W_hh"], dtype=np.float32)
    bias = (
        np.asarray(inputs["b_ih"], dtype=np.float32)
        + np.asarray(inputs["b_hh"], dtype=np.float32)
    )
    # permute gate order (i, f, g, o) -> (i, f, o, g) so one tanh covers all
    # gates; i/f/o rows are halved so tanh(x/2) gives 2*sigmoid(x)-1
    perm = np.concatenate(
        [np.arange(0, 2 * H), np.arange(3 * H, 4 * H), np.arange(2 * H, 3 * H)]
    )
    W_ih = W_ih[perm]
    W_hh = W_hh[perm]
    bias = bias[perm]
    ifo = 3 * H  # first 768 rows are i, f, o
    W_ih[:ifo] *= 0.5
    bias[:ifo] *= 0.5
    # hidden state is stored as 2h: all W_hh gets an extra 0.5
    W_hh[:ifo] *= 0.25
    W_hh[ifo:] *= 0.5
    W_lin = np.asarray(inputs["W_lin"], dtype=np.float32)
    b_lin = np.asarray(inputs["b_lin"], dtype=np.float32)

    # embedding lookup on host, transposed to [E, (t, b)] per batch group
    x_bt = emb[target.astype(np.int64)].astype(bf)                # (B, T, E)
    wihT = np.ascontiguousarray(W_ih.T.astype(bf))                # (E, 4H)
    whhT = np.ascontiguousarray(
        W_hh.T.reshape(2, 128, G4H).astype(bf)
    )                                                             # (2,128,4H)
    biasT = np.ascontiguousarray(bias.reshape(NCH, 128).T)        # (128, NCH)
    wlinT_full = W_lin.T.copy()                                   # (512, V)
    wlinT_full[:H] *= 0.5                                         # z01 = 2h
    wlinT_full = wlinT_full.astype(bf)

    in_maps = []
    for i in range(NCORES):
        bg = i % NBG
        vh = i // NBG
        sl = slice(bg * BL, (bg + 1) * BL)
        vsl = slice(vh * VL, (vh + 1) * VL)
        xT = np.ascontiguousarray(
            x_bt[sl].transpose(2, 1, 0).reshape(E, BT)
        )  # [E, (t, b)]
        enc_i = enc[sl]                                           # (BL, S, H)
        enc_sbh = np.ascontiguousarray(
            enc_i.transpose(1, 0, 2).astype(bf)
        )                                                         # (S, BL, H)
        encT = np.ascontiguousarray(
            enc_i.transpose(2, 0, 1).reshape(2, 128, BL, S).astype(bf)
        )                                                         # (2,128,BL,S)
        h0T = np.ascontiguousarray(
            (2.0 * h0[sl]).T.reshape(2, 128, BL).transpose(1, 0, 2)
        )
        c0T = np.ascontiguousarray(
            (2.0 * c0[sl]).T.reshape(2, 128, BL).transpose(1, 0, 2)
        )
        wlinT = np.ascontiguousarray(
            wlinT_full[:, vsl].reshape(4, 128, VL)
        )                                                         # (4,128,VL)
        in_maps.append(
            {
                "xT": xT,
                "enc": enc_sbh,
                "encT": encT,
                "h0T": h0T,
                "c0T": c0T,
                "lens": np.ascontiguousarray(lens[sl].astype(np.int32)),
                "biasT": biasT,
                "wihT": wihT,
                "whhT": whhT,
                "wlinT": wlinT,
            }
        )
    return in_maps, b_lin


LAST_RESULTS = None


def _install_ntff_shim():
    """Provide antenv.axon_hooks if the image's antenv lacks it, so
    trace=True/BASS_TRACE=1 can capture NTFF profiles under axon."""
    import sys
    import types

    try:
        from antenv.axon_hooks import get_axon_ntff_profile_hook  # noqa: F401

        return
    except ImportError:
        pass
    try:
        from trn_agent_boot.trn_boot import _ntff_profile_via_ctypes

        hook = _ntff_profile_via_ctypes("/opt/axon/libaxon_pjrt.so")
        m = types.ModuleType("antenv.axon_hooks")
        m.get_axon_ntff_profile_hook = lambda: hook
        m.set_axon_ntff_profile_hook = lambda h: None
        sys.modules["antenv.axon_hooks"] = m
    except Exception:
        pass


def kernel(**inputs):
    global LAST_RESULTS
    _install_ntff_shim()
    if "nc" not in _CACHE:
        _CACHE["nc"] = _build()
    nc = _CACHE["nc"]
    in_maps, b_lin = _prep_inputs(inputs)
    res = run_bass_kernel_spmd(nc, in_maps, core_ids=list(range(NCORES)))
    LAST_RESULTS = res
    out = np.empty((B, T, V), dtype=np.float32)
    for i in range(NCORES):
        bg = i % NBG
        vh = i // NBG
        vsl = slice(vh * VL, (vh + 1) * VL)
        # logits rows are t-major: row = t*BL + b
        lg = res.results[i]["logits"].reshape(T, BL, VL).transpose(1, 0, 2)
        out[bg * BL : (bg + 1) * BL, :, vsl] = lg
        out[bg * BL : (bg + 1) * BL, :, vsl] += b_lin[vsl]
    return out
